# revision 18
# baseline (speedup 1.0000x reference)
"""Bahdanau-attention kernel for 8 TRN2 NeuronCores.

Reference computation (B=32, S=2048, H=1024):
    eo   = encoder_outputs.transpose(1,0,2)            # [B,S,H]
    z    = hidden @ W[:, :H].T + eo @ W[:, H:].T + b   # [B,S,H]  (split concat)
    s    = tanh(z)
    sc   = einsum('bsh,h->bs', s, v)
    sc   = where(mask, -1e9, sc); softmax over S       # [B,1,S]

Sharding: data-parallel over batch, 4 batches per core, no collectives.
Per core: z_eo = We @ eo_b^T as [h, s] tiles on TensorE (bf16, fp32 psum),
tanh + per-(h,b) bias fused on ScalarE, v-weighted accumulate on VectorE,
partition-reduce via ones-matmul, chunked masked softmax on-device.

Softmax skips the max-subtraction: |score| <= sum|v| ~ 16, so exp() stays
comfortably inside fp32 range, and masked lanes see exp(-1e30) == 0.
"""

import sys

if "/opt/trn_rl_repo" not in sys.path:
    sys.path.insert(0, "/opt/trn_rl_repo")

import numpy as np

B, S, H = 32, 2048, 1024
NCORES = 8
BL = B // NCORES          # batches per core = 4
P = 128                   # partitions
KT = H // P               # k-tiles over the contraction dim = 8
HT = H // P               # h-tiles over the attn output dim = 8
ST = 512                  # s-tile (psum bank width in fp32)
NS = S // ST              # s-tiles per batch = 4

_compiled_nc = None


def _build():
    import concourse.mybir as mybir
    from concourse import tile, bacc
    from concourse.tile import add_dep_helper

    f32 = mybir.dt.float32
    bf16 = mybir.dt.bfloat16
    u8 = mybir.dt.uint8
    AF = mybir.ActivationFunctionType
    ALU = mybir.AluOpType
    AX = mybir.AxisListType

    nc = bacc.Bacc("TRN2", target_bir_lowering=False, debug=False,
                   num_devices=NCORES)

    eoT = nc.dram_tensor("eoT", [BL, H, S], f32, kind="ExternalInput")
    wT = nc.dram_tensor("wT", [2 * H, H], bf16, kind="ExternalInput")
    hTr = nc.dram_tensor("hTr", [P, KT, BL], bf16, kind="ExternalInput")
    biasr = nc.dram_tensor("biasr", [P, HT], f32, kind="ExternalInput")
    vr = nc.dram_tensor("vr", [P, HT], f32, kind="ExternalInput")
    mask = nc.dram_tensor("mask", [BL, S], u8, kind="ExternalInput")
    out = nc.dram_tensor("out", [BL, S], f32, kind="ExternalOutput")

    with tile.TileContext(nc) as tc:
        with (
            tc.tile_pool(name="const", bufs=1) as const,
            tc.tile_pool(name="eo", bufs=5) as eo_pool,
            tc.tile_pool(name="tpool", bufs=4) as t_pool,
            tc.tile_pool(name="tvpool", bufs=4) as tv_pool,
            tc.tile_pool(name="accpool", bufs=3) as acc_pool,
            tc.tile_pool(name="scpool", bufs=2) as sc_pool,
            tc.tile_pool(name="mskpool", bufs=2) as msk_pool,
            tc.tile_pool(name="psz", bufs=6, space="PSUM") as psum_z,
            tc.tile_pool(name="pss", bufs=2, space="PSUM") as psum_s,
        ):
            # --- tiny consts land first (HWDGE), gate ScalarE/pre ---
            hT_sb = const.tile([P, KT, BL], bf16)
            nc.sync.dma_start(hT_sb[:], hTr[:, :, :])
            bias_sb = const.tile([P, HT], f32)
            nc.sync.dma_start(bias_sb[:], biasr[:, :])
            v_sb = const.tile([P, HT], f32)
            nc.sync.dma_start(v_sb[:], vr[:, :])
            mask_row = const.tile([1, BL * S], u8)
            nc.sync.dma_start(mask_row[:],
                              mask.rearrange("b s -> (b s)")[None, :])

            ones_sb = const.tile([P, 1], bf16)
            nc.any.memset(ones_sb[:], 1.0)
            junk = const.tile([P, ST], bf16)
            nc.vector.tensor_copy(junk[:, 0:1], ones_sb[:])

            # big DMAs all on the SWDGE queue: strict FIFO means the
            # weights beat the eo prefetches to HBM bandwidth.
            # weights ride the HWDGE ring (sub-us first byte, FIFO among
            # themselves); the first eo tile rides SWDGE concurrently.
            # Later eo prefetches chain behind we0 so the head window only
            # ever has ~2 streams splitting HBM bandwidth.
            wh_sb = const.tile([P, KT, H], bf16)
            nc.sync.dma_start(
                wh_sb[:], wT[0:H, :].rearrange("(kk p) h -> p kk h", p=P))
            eo_first = eo_pool.tile([P, KT, ST], bf16, tag="eo")
            nc.gpsimd.dma_start(
                eo_first[:],
                eoT[0, :, 0:ST].rearrange("(kk p) s -> p kk s", p=P))
            we_sb = const.tile([P, KT, H], bf16)
            d_we0 = nc.sync.dma_start(
                we_sb[:, :, 0:H // 2],
                wT[H:2 * H, 0:H // 2].rearrange("(kk p) h -> p kk h", p=P))
            nc.sync.dma_start(
                we_sb[:, :, H // 2:H],
                wT[H:2 * H, H // 2:H].rearrange("(kk p) h -> p kk h", p=P))
            _dma_chain = [d_we0]

            mneg_row = const.tile([1, BL * S], f32)
            nc.vector.tensor_scalar(mneg_row[:], mask_row[:], -1e30, None,
                                    ALU.mult)

            # PE warmup: dummy matmuls ride out the HAM cold window while
            # the weight/eo DMAs stream in.
            wps = psum_z.tile([P, ST], f32, tag="psz")
            for w in range(40):
                nc.tensor.matmul(wps[:], junk[:, 0:P], junk[:],
                                 start=(w == 0), stop=(w == 39),
                                 skip_group_check=True)

            # pre[h, b] = (hidden @ Wh^T)[b, h] + bias[h], h on partitions.
            pre_sb = const.tile([P, HT * BL], f32)
            for hh in range(HT):
                ps = psum_z.tile([P, ST], f32, tag="psz")
                for kk in range(KT):
                    nc.tensor.matmul(
                        ps[:, :BL],
                        wh_sb[:, kk, hh * P:(hh + 1) * P],
                        hT_sb[:, kk, :],
                        start=(kk == 0), stop=(kk == KT - 1))
                nc.scalar.activation(pre_sb[:, hh * BL:(hh + 1) * BL],
                                     ps[:, :BL], AF.Identity,
                                     bias=bias_sb[:, hh:hh + 1])

            # second warmup burst: keeps the PE busy between `pre` and the
            # arrival of the first eo tile
            wps2 = psum_z.tile([P, ST], f32, tag="psz")
            for w in range(8):
                nc.tensor.matmul(wps2[:], junk[:, 0:P], junk[:],
                                 start=(w == 0), stop=(w == 7),
                                 skip_group_check=True)

            e_sb = const.tile([BL, S], f32)
            red_row = const.tile([1, BL * NS], f32)
            psums4 = const.tile([BL, NS], f32)

            def flush_score(pend):
                if pend is None:
                    return
                acc_p, b_p, si_p = pend
                # partition-reduce on GpSimd (TensorE stays on the main GEMM)
                sc_r = sc_pool.tile([1, ST], f32, tag="scr")
                nc.gpsimd.tensor_reduce(sc_r[:], acc_p[:], axis=AX.C,
                                        op=ALU.add)
                # mask + exp on the partition-0 row, partial sum via accum
                sc_m = sc_pool.tile([1, ST], f32, tag="sc")
                off = b_p * S + si_p * ST
                nc.vector.tensor_tensor(sc_m[:], sc_r[:],
                                        mneg_row[:, off:off + ST], ALU.add)
                e_row = msk_pool.tile([1, ST], f32, tag="m")
                idx = b_p * NS + si_p
                nc.scalar.activation(e_row[:], sc_m[:], AF.Exp,
                                     accum_out=red_row[:, idx:idx + 1])
                nc.sync.dma_start(
                    e_sb[b_p:b_p + 1, si_p * ST:(si_p + 1) * ST], e_row[:])
                # scatter this tile's partial sum to its batch partition now
                nc.sync.dma_start(psums4[b_p:b_p + 1, si_p:si_p + 1],
                                  red_row[:, idx:idx + 1])

            pending = None
            # si-major so each score column-chunk [BL, ST] completes early
            # and its softmax piece overlaps the next chunk's matmuls.
            for si in range(NS):
                for b in range(BL):
                    if b == 0 and si == 0:
                        eo_sb = eo_first
                    else:
                        eo_sb = eo_pool.tile([P, KT, ST], bf16, tag="eo")
                        d_eo = nc.gpsimd.dma_start(
                            eo_sb[:],
                            eoT[b, :, si * ST:(si + 1) * ST].rearrange(
                                "(kk p) s -> p kk s", p=P))
                        if len(_dma_chain) < 4:
                            add_dep_helper(d_eo.ins, _dma_chain[-1].ins, True,
                                           "serial head dma")
                            _dma_chain.append(d_eo)
                    acc = acc_pool.tile([P, ST], bf16, tag="acc")
                    for hh in range(HT):
                        ps = psum_z.tile([P, ST], f32, tag="psz")
                        for kk in range(KT):
                            nc.tensor.matmul(
                                ps[:],
                                we_sb[:, kk, hh * P:(hh + 1) * P],
                                eo_sb[:, kk, :],
                                start=(kk == 0), stop=(kk == KT - 1))
                        if hh == 3:
                            flush_score(pending)
                            pending = None
                        t_sb = t_pool.tile([P, ST], bf16, tag="t")
                        nc.scalar.activation(
                            t_sb[:], ps[:], AF.Tanh,
                            bias=pre_sb[:, hh * BL + b:hh * BL + b + 1])
                        if hh == 0:
                            nc.vector.tensor_scalar(acc[:], t_sb[:],
                                                    v_sb[:, 0:1], None,
                                                    ALU.mult)
                        else:
                            tv = tv_pool.tile([P, ST], bf16, tag="tv")
                            nc.vector.tensor_scalar(tv[:], t_sb[:],
                                                    v_sb[:, hh:hh + 1], None,
                                                    ALU.mult)
                            nc.vector.tensor_tensor(acc[:], acc[:], tv[:],
                                                    ALU.add)
                    pending = (acc, b, si)

            flush_score(pending)

            # per-batch totals and reciprocal, already partition-aligned
            rinv4 = const.tile([BL, 1], f32)
            nc.vector.reduce_sum(rinv4[:], psums4[:], axis=AX.X)
            nc.vector.reciprocal(rinv4[:], rinv4[:])
            # normalize + store in two chunks so the DMA overlaps the mul
            o_sb = const.tile([BL, S], f32)
            for ci in range(2):
                cs = slice(ci * (S // 2), (ci + 1) * (S // 2))
                nc.vector.tensor_scalar(o_sb[:, cs], e_sb[:, cs], rinv4[:],
                                        None, ALU.mult)
                nc.sync.dma_start(out[:, cs], o_sb[:, cs])

    nc.compile()
    return nc


def _get_nc():
    global _compiled_nc
    if _compiled_nc is None:
        _compiled_nc = _build()
    return _compiled_nc


def _make_in_maps(hidden, encoder_outputs, encoder_mask, W, b, v):
    import ml_dtypes

    bf16 = ml_dtypes.bfloat16
    hidden = np.asarray(hidden, dtype=np.float32)
    encoder_outputs = np.asarray(encoder_outputs, dtype=np.float32)
    W = np.asarray(W, dtype=np.float32)
    b = np.asarray(b, dtype=np.float32)
    v = np.asarray(v, dtype=np.float32)
    mask_u8 = np.asarray(encoder_mask).reshape(B, S).astype(np.uint8)

    # [S, B, H] -> [B, H, S] so the contraction dim lands on partitions
    eoT = np.ascontiguousarray(encoder_outputs.transpose(1, 2, 0))
    wT = np.ascontiguousarray(W.T).astype(bf16)         # [2H, H]
    bias_r = np.ascontiguousarray(b.reshape(HT, P).T)   # [P, HT]
    v_r = np.ascontiguousarray(v.reshape(HT, P).T)      # [P, HT]

    in_maps = []
    for c in range(NCORES):
        bs = slice(c * BL, (c + 1) * BL)
        h_c = hidden[bs]                                # [BL, H]
        hT_r = np.ascontiguousarray(
            h_c.T.reshape(KT, P, BL).transpose(1, 0, 2)).astype(bf16)
        in_maps.append({
            "eoT": eoT[bs],
            "wT": wT,
            "hTr": hT_r,
            "biasr": bias_r,
            "vr": v_r,
            "mask": mask_u8[bs],
        })
    return in_maps


def run(hidden, encoder_outputs, encoder_mask, W, b, v, trace=False):
    from concourse.bass_utils import run_bass_kernel_spmd

    nc = _get_nc()
    in_maps = _make_in_maps(hidden, encoder_outputs, encoder_mask, W, b, v)
    res = run_bass_kernel_spmd(nc, in_maps, core_ids=list(range(NCORES)),
                               trace=trace)
    out = np.concatenate([res.results[c]["out"] for c in range(NCORES)],
                         axis=0)
    return out.reshape(B, 1, S).astype(np.float32), res


def kernel(hidden, encoder_outputs, encoder_mask, W, b, v):
    out, _ = run(hidden, encoder_outputs, encoder_mask, W, b, v, trace=False)
    return out


# revision 19
# speedup vs baseline: 4.4398x; 4.4398x over previous
"""Bahdanau-attention kernel for 8 TRN2 NeuronCores.

Reference computation (B=32, S=2048, H=1024):
    eo   = encoder_outputs.transpose(1,0,2)            # [B,S,H]
    z    = hidden @ W[:, :H].T + eo @ W[:, H:].T + b   # [B,S,H]  (split concat)
    s    = tanh(z)
    sc   = einsum('bsh,h->bs', s, v)
    sc   = where(mask, -1e9, sc); softmax over S       # [B,1,S]

Sharding: data-parallel over batch, 4 batches per core, no collectives.
Per core: z_eo = We @ eo_b^T as [h, s] tiles on TensorE (bf16, fp32 psum),
tanh + per-(h,b) bias fused on ScalarE, v-weighted accumulate on VectorE,
partition-reduce via ones-matmul, chunked masked softmax on-device.

Softmax skips the max-subtraction: |score| <= sum|v| ~ 16, so exp() stays
comfortably inside fp32 range, and masked lanes see exp(-1e30) == 0.
"""

import sys

if "/opt/trn_rl_repo" not in sys.path:
    sys.path.insert(0, "/opt/trn_rl_repo")

import numpy as np

B, S, H = 32, 2048, 1024
NCORES = 8
BL = B // NCORES          # batches per core = 4
P = 128                   # partitions
KT = H // P               # k-tiles over the contraction dim = 8
HT = H // P               # h-tiles over the attn output dim = 8
ST = 512                  # s-tile (psum bank width in fp32)
NS = S // ST              # s-tiles per batch = 4

_compiled_nc = None


def _build():
    import concourse.mybir as mybir
    from concourse import tile, bacc
    from concourse.tile import add_dep_helper

    f32 = mybir.dt.float32
    bf16 = mybir.dt.bfloat16
    u8 = mybir.dt.uint8
    AF = mybir.ActivationFunctionType
    ALU = mybir.AluOpType
    AX = mybir.AxisListType

    nc = bacc.Bacc("TRN2", target_bir_lowering=False, debug=False,
                   num_devices=NCORES)

    eoT = nc.dram_tensor("eoT", [BL, H, S], f32, kind="ExternalInput")
    wT = nc.dram_tensor("wT", [2 * H, H], bf16, kind="ExternalInput")
    hTr = nc.dram_tensor("hTr", [P, KT, BL], bf16, kind="ExternalInput")
    biasr = nc.dram_tensor("biasr", [P, HT], f32, kind="ExternalInput")
    vr = nc.dram_tensor("vr", [P, HT], f32, kind="ExternalInput")
    mask = nc.dram_tensor("mask", [BL, S], u8, kind="ExternalInput")
    out = nc.dram_tensor("out", [BL, S], f32, kind="ExternalOutput")

    with tile.TileContext(nc) as tc:
        with (
            tc.tile_pool(name="const", bufs=1) as const,
            tc.tile_pool(name="eo", bufs=5) as eo_pool,
            tc.tile_pool(name="tpool", bufs=4) as t_pool,
            tc.tile_pool(name="tvpool", bufs=4) as tv_pool,
            tc.tile_pool(name="accpool", bufs=3) as acc_pool,
            tc.tile_pool(name="scpool", bufs=2) as sc_pool,
            tc.tile_pool(name="mskpool", bufs=2) as msk_pool,
            tc.tile_pool(name="psz", bufs=6, space="PSUM") as psum_z,
            tc.tile_pool(name="pss", bufs=2, space="PSUM") as psum_s,
        ):
            # --- tiny consts land first (HWDGE), gate ScalarE/pre ---
            hT_sb = const.tile([P, KT, BL], bf16)
            nc.sync.dma_start(hT_sb[:], hTr[:, :, :])
            bias_sb = const.tile([P, HT], f32)
            nc.sync.dma_start(bias_sb[:], biasr[:, :])
            v_sb = const.tile([P, HT], f32)
            nc.sync.dma_start(v_sb[:], vr[:, :])
            mask_row = const.tile([1, BL * S], u8)
            nc.sync.dma_start(mask_row[:],
                              mask.rearrange("b s -> (b s)")[None, :])

            ones_sb = const.tile([P, 1], bf16)
            nc.any.memset(ones_sb[:], 1.0)
            junk = const.tile([P, ST], bf16)
            nc.vector.tensor_copy(junk[:, 0:1], ones_sb[:])

            # big DMAs all on the SWDGE queue: strict FIFO means the
            # weights beat the eo prefetches to HBM bandwidth.
            # weights ride the HWDGE ring (sub-us first byte, FIFO among
            # themselves); the first eo tile rides SWDGE concurrently.
            # Later eo prefetches chain behind we0 so the head window only
            # ever has ~2 streams splitting HBM bandwidth.
            wh_sb = const.tile([P, KT, H], bf16)
            nc.sync.dma_start(
                wh_sb[:], wT[0:H, :].rearrange("(kk p) h -> p kk h", p=P))
            eo_first = eo_pool.tile([P, KT, ST], bf16, tag="eo")
            nc.gpsimd.dma_start(
                eo_first[:],
                eoT[0, :, 0:ST].rearrange("(kk p) s -> p kk s", p=P))
            we_sb = const.tile([P, KT, H], bf16)
            d_we0 = nc.sync.dma_start(
                we_sb[:, :, 0:H // 2],
                wT[H:2 * H, 0:H // 2].rearrange("(kk p) h -> p kk h", p=P))
            nc.sync.dma_start(
                we_sb[:, :, H // 2:H],
                wT[H:2 * H, H // 2:H].rearrange("(kk p) h -> p kk h", p=P))
            _dma_chain = [d_we0]

            mneg_row = const.tile([1, BL * S], f32)
            nc.vector.tensor_scalar(mneg_row[:], mask_row[:], -1e30, None,
                                    ALU.mult)

            # PE warmup: dummy matmuls ride out the HAM cold window while
            # the weight/eo DMAs stream in.
            wps = psum_z.tile([P, ST], f32, tag="psz")
            for w in range(40):
                nc.tensor.matmul(wps[:], junk[:, 0:P], junk[:],
                                 start=(w == 0), stop=(w == 39),
                                 skip_group_check=True)

            # pre[h, b] = (hidden @ Wh^T)[b, h] + bias[h], h on partitions.
            pre_sb = const.tile([P, HT * BL], f32)
            for hh in range(HT):
                ps = psum_z.tile([P, ST], f32, tag="psz")
                for kk in range(KT):
                    nc.tensor.matmul(
                        ps[:, :BL],
                        wh_sb[:, kk, hh * P:(hh + 1) * P],
                        hT_sb[:, kk, :],
                        start=(kk == 0), stop=(kk == KT - 1))
                nc.scalar.activation(pre_sb[:, hh * BL:(hh + 1) * BL],
                                     ps[:, :BL], AF.Identity,
                                     bias=bias_sb[:, hh:hh + 1])

            # second warmup burst: keeps the PE busy between `pre` and the
            # arrival of the first eo tile
            wps2 = psum_z.tile([P, ST], f32, tag="psz")
            for w in range(8):
                nc.tensor.matmul(wps2[:], junk[:, 0:P], junk[:],
                                 start=(w == 0), stop=(w == 7),
                                 skip_group_check=True)

            e_sb = const.tile([BL, S], f32)
            red_row = const.tile([1, BL * NS], f32)
            psums4 = const.tile([BL, NS], f32)

            def flush_score(pend):
                if pend is None:
                    return
                acc_p, b_p, si_p = pend
                pssc = psum_s.tile([P, ST], f32, tag="pss")
                nc.tensor.matmul(pssc[:1], ones_sb[:], acc_p[:],
                                 start=True, stop=True)
                # mask + exp on the partition-0 row, partial sum via accum
                sc_m = sc_pool.tile([1, ST], f32, tag="sc")
                off = b_p * S + si_p * ST
                nc.vector.tensor_tensor(sc_m[:], pssc[:1],
                                        mneg_row[:, off:off + ST], ALU.add)
                e_row = msk_pool.tile([1, ST], f32, tag="m")
                idx = b_p * NS + si_p
                nc.scalar.activation(e_row[:], sc_m[:], AF.Exp,
                                     accum_out=red_row[:, idx:idx + 1])
                nc.sync.dma_start(
                    e_sb[b_p:b_p + 1, si_p * ST:(si_p + 1) * ST], e_row[:])
                # scatter this tile's partial sum to its batch partition now
                nc.sync.dma_start(psums4[b_p:b_p + 1, si_p:si_p + 1],
                                  red_row[:, idx:idx + 1])

            pending = None
            # si-major so each score column-chunk [BL, ST] completes early
            # and its softmax piece overlaps the next chunk's matmuls.
            for si in range(NS):
                for b in range(BL):
                    if b == 0 and si == 0:
                        eo_sb = eo_first
                    else:
                        eo_sb = eo_pool.tile([P, KT, ST], bf16, tag="eo")
                        d_eo = nc.gpsimd.dma_start(
                            eo_sb[:],
                            eoT[b, :, si * ST:(si + 1) * ST].rearrange(
                                "(kk p) s -> p kk s", p=P))
                        if len(_dma_chain) < 4:
                            add_dep_helper(d_eo.ins, _dma_chain[-1].ins, True,
                                           "serial head dma")
                            _dma_chain.append(d_eo)
                    acc = acc_pool.tile([P, ST], bf16, tag="acc")
                    for hh in range(HT):
                        ps = psum_z.tile([P, ST], f32, tag="psz")
                        for kk in range(KT):
                            nc.tensor.matmul(
                                ps[:],
                                we_sb[:, kk, hh * P:(hh + 1) * P],
                                eo_sb[:, kk, :],
                                start=(kk == 0), stop=(kk == KT - 1))
                        if hh == 3:
                            flush_score(pending)
                            pending = None
                        t_sb = t_pool.tile([P, ST], bf16, tag="t")
                        nc.scalar.activation(
                            t_sb[:], ps[:], AF.Tanh,
                            bias=pre_sb[:, hh * BL + b:hh * BL + b + 1])
                        if hh == 0:
                            nc.vector.tensor_scalar(acc[:], t_sb[:],
                                                    v_sb[:, 0:1], None,
                                                    ALU.mult)
                        else:
                            tv = tv_pool.tile([P, ST], bf16, tag="tv")
                            nc.vector.tensor_scalar(tv[:], t_sb[:],
                                                    v_sb[:, hh:hh + 1], None,
                                                    ALU.mult)
                            nc.vector.tensor_tensor(acc[:], acc[:], tv[:],
                                                    ALU.add)
                    pending = (acc, b, si)

            flush_score(pending)

            # per-batch totals and reciprocal, already partition-aligned
            rinv4 = const.tile([BL, 1], f32)
            nc.vector.reduce_sum(rinv4[:], psums4[:], axis=AX.X)
            nc.vector.reciprocal(rinv4[:], rinv4[:])
            # normalize + store in two chunks so the DMA overlaps the mul
            o_sb = const.tile([BL, S], f32)
            for ci in range(2):
                cs = slice(ci * (S // 2), (ci + 1) * (S // 2))
                nc.vector.tensor_scalar(o_sb[:, cs], e_sb[:, cs], rinv4[:],
                                        None, ALU.mult)
                nc.sync.dma_start(out[:, cs], o_sb[:, cs])

    nc.compile()
    return nc


def _get_nc():
    global _compiled_nc
    if _compiled_nc is None:
        _compiled_nc = _build()
    return _compiled_nc


def _make_in_maps(hidden, encoder_outputs, encoder_mask, W, b, v):
    import ml_dtypes

    bf16 = ml_dtypes.bfloat16
    hidden = np.asarray(hidden, dtype=np.float32)
    encoder_outputs = np.asarray(encoder_outputs, dtype=np.float32)
    W = np.asarray(W, dtype=np.float32)
    b = np.asarray(b, dtype=np.float32)
    v = np.asarray(v, dtype=np.float32)
    mask_u8 = np.asarray(encoder_mask).reshape(B, S).astype(np.uint8)

    # [S, B, H] -> [B, H, S] so the contraction dim lands on partitions
    eoT = np.ascontiguousarray(encoder_outputs.transpose(1, 2, 0))
    wT = np.ascontiguousarray(W.T).astype(bf16)         # [2H, H]
    bias_r = np.ascontiguousarray(b.reshape(HT, P).T)   # [P, HT]
    v_r = np.ascontiguousarray(v.reshape(HT, P).T)      # [P, HT]

    in_maps = []
    for c in range(NCORES):
        bs = slice(c * BL, (c + 1) * BL)
        h_c = hidden[bs]                                # [BL, H]
        hT_r = np.ascontiguousarray(
            h_c.T.reshape(KT, P, BL).transpose(1, 0, 2)).astype(bf16)
        in_maps.append({
            "eoT": eoT[bs],
            "wT": wT,
            "hTr": hT_r,
            "biasr": bias_r,
            "vr": v_r,
            "mask": mask_u8[bs],
        })
    return in_maps


def run(hidden, encoder_outputs, encoder_mask, W, b, v, trace=False):
    from concourse.bass_utils import run_bass_kernel_spmd

    nc = _get_nc()
    in_maps = _make_in_maps(hidden, encoder_outputs, encoder_mask, W, b, v)
    res = run_bass_kernel_spmd(nc, in_maps, core_ids=list(range(NCORES)),
                               trace=trace)
    out = np.concatenate([res.results[c]["out"] for c in range(NCORES)],
                         axis=0)
    return out.reshape(B, 1, S).astype(np.float32), res


def kernel(hidden, encoder_outputs, encoder_mask, W, b, v):
    out, _ = run(hidden, encoder_outputs, encoder_mask, W, b, v, trace=False)
    return out


# revision 20
# speedup vs baseline: 4.4720x; 1.0072x over previous
"""Bahdanau-attention kernel for 8 TRN2 NeuronCores.

Reference computation (B=32, S=2048, H=1024):
    eo   = encoder_outputs.transpose(1,0,2)            # [B,S,H]
    z    = hidden @ W[:, :H].T + eo @ W[:, H:].T + b   # [B,S,H]  (split concat)
    s    = tanh(z)
    sc   = einsum('bsh,h->bs', s, v)
    sc   = where(mask, -1e9, sc); softmax over S       # [B,1,S]

Sharding: data-parallel over batch, 4 batches per core, no collectives.
Per core: z_eo = We @ eo_b^T as [h, s] tiles on TensorE (bf16, fp32 psum),
tanh + per-(h,b) bias fused on ScalarE, v-weighted accumulate on VectorE,
partition-reduce via ones-matmul, chunked masked softmax on-device.

Softmax skips the max-subtraction: |score| <= sum|v| ~ 16, so exp() stays
comfortably inside fp32 range, and masked lanes see exp(-1e30) == 0.
"""

import sys

if "/opt/trn_rl_repo" not in sys.path:
    sys.path.insert(0, "/opt/trn_rl_repo")

import numpy as np

B, S, H = 32, 2048, 1024
NCORES = 8
BL = B // NCORES          # batches per core = 4
P = 128                   # partitions
KT = H // P               # k-tiles over the contraction dim = 8
HT = H // P               # h-tiles over the attn output dim = 8
ST = 512                  # s-tile (psum bank width in fp32)
NS = S // ST              # s-tiles per batch = 4

_compiled_nc = None


def _build():
    import concourse.mybir as mybir
    from concourse import tile, bacc
    from concourse.tile import add_dep_helper

    f32 = mybir.dt.float32
    bf16 = mybir.dt.bfloat16
    u8 = mybir.dt.uint8
    AF = mybir.ActivationFunctionType
    ALU = mybir.AluOpType
    AX = mybir.AxisListType

    nc = bacc.Bacc("TRN2", target_bir_lowering=False, debug=False,
                   num_devices=NCORES)

    eoT = nc.dram_tensor("eoT", [BL, H, S], bf16, kind="ExternalInput")
    wT = nc.dram_tensor("wT", [2 * H, H], bf16, kind="ExternalInput")
    hTr = nc.dram_tensor("hTr", [P, KT, BL], bf16, kind="ExternalInput")
    biasr = nc.dram_tensor("biasr", [P, HT], f32, kind="ExternalInput")
    vr = nc.dram_tensor("vr", [P, HT], f32, kind="ExternalInput")
    mask = nc.dram_tensor("mask", [BL, S], u8, kind="ExternalInput")
    out = nc.dram_tensor("out", [BL, S], f32, kind="ExternalOutput")

    with tile.TileContext(nc) as tc:
        with (
            tc.tile_pool(name="const", bufs=1) as const,
            tc.tile_pool(name="eo", bufs=5) as eo_pool,
            tc.tile_pool(name="tpool", bufs=4) as t_pool,
            tc.tile_pool(name="tvpool", bufs=4) as tv_pool,
            tc.tile_pool(name="accpool", bufs=3) as acc_pool,
            tc.tile_pool(name="scpool", bufs=2) as sc_pool,
            tc.tile_pool(name="mskpool", bufs=2) as msk_pool,
            tc.tile_pool(name="psz", bufs=6, space="PSUM") as psum_z,
            tc.tile_pool(name="pss", bufs=2, space="PSUM") as psum_s,
        ):
            # --- tiny consts land first (HWDGE), gate ScalarE/pre ---
            hT_sb = const.tile([P, KT, BL], bf16)
            nc.sync.dma_start(hT_sb[:], hTr[:, :, :])
            bias_sb = const.tile([P, HT], f32)
            nc.sync.dma_start(bias_sb[:], biasr[:, :])
            v_sb = const.tile([P, HT], f32)
            nc.sync.dma_start(v_sb[:], vr[:, :])
            mask_row = const.tile([1, BL * S], u8)
            nc.sync.dma_start(mask_row[:],
                              mask.rearrange("b s -> (b s)")[None, :])

            ones_sb = const.tile([P, 1], bf16)
            nc.any.memset(ones_sb[:], 1.0)
            junk = const.tile([P, ST], bf16)
            nc.vector.tensor_copy(junk[:, 0:1], ones_sb[:])

            # big DMAs all on the SWDGE queue: strict FIFO means the
            # weights beat the eo prefetches to HBM bandwidth.
            # weights ride the HWDGE ring (sub-us first byte, FIFO among
            # themselves); the first eo tile rides SWDGE concurrently.
            # Later eo prefetches chain behind we0 so the head window only
            # ever has ~2 streams splitting HBM bandwidth.
            wh_sb = const.tile([P, KT, H], bf16)
            nc.sync.dma_start(
                wh_sb[:], wT[0:H, :].rearrange("(kk p) h -> p kk h", p=P))
            eo_first = eo_pool.tile([P, KT, ST], bf16, tag="eo")
            nc.gpsimd.dma_start(
                eo_first[:],
                eoT[0, :, 0:ST].rearrange("(kk p) s -> p kk s", p=P))
            we_sb = const.tile([P, KT, H], bf16)
            d_we0 = nc.sync.dma_start(
                we_sb[:, :, 0:H // 2],
                wT[H:2 * H, 0:H // 2].rearrange("(kk p) h -> p kk h", p=P))
            nc.sync.dma_start(
                we_sb[:, :, H // 2:H],
                wT[H:2 * H, H // 2:H].rearrange("(kk p) h -> p kk h", p=P))
            _dma_chain = [d_we0]

            mneg_row = const.tile([1, BL * S], f32)
            nc.vector.tensor_scalar(mneg_row[:], mask_row[:], -1e30, None,
                                    ALU.mult)

            # PE warmup: dummy matmuls ride out the HAM cold window while
            # the weight/eo DMAs stream in.
            wps = psum_z.tile([P, ST], f32, tag="psz")
            for w in range(40):
                nc.tensor.matmul(wps[:], junk[:, 0:P], junk[:],
                                 start=(w == 0), stop=(w == 39),
                                 skip_group_check=True)

            # pre[h, b] = (hidden @ Wh^T)[b, h] + bias[h], h on partitions.
            pre_sb = const.tile([P, HT * BL], f32)
            for hh in range(HT):
                ps = psum_z.tile([P, ST], f32, tag="psz")
                for kk in range(KT):
                    nc.tensor.matmul(
                        ps[:, :BL],
                        wh_sb[:, kk, hh * P:(hh + 1) * P],
                        hT_sb[:, kk, :],
                        start=(kk == 0), stop=(kk == KT - 1))
                nc.scalar.activation(pre_sb[:, hh * BL:(hh + 1) * BL],
                                     ps[:, :BL], AF.Identity,
                                     bias=bias_sb[:, hh:hh + 1])

            # second warmup burst: keeps the PE busy between `pre` and the
            # arrival of the first eo tile
            wps2 = psum_z.tile([P, ST], f32, tag="psz")
            for w in range(8):
                nc.tensor.matmul(wps2[:], junk[:, 0:P], junk[:],
                                 start=(w == 0), stop=(w == 7),
                                 skip_group_check=True)

            e_sb = const.tile([BL, S], f32)
            red_row = const.tile([1, BL * NS], f32)
            psums4 = const.tile([BL, NS], f32)

            def flush_score(pend):
                if pend is None:
                    return
                acc_p, b_p, si_p = pend
                pssc = psum_s.tile([P, ST], f32, tag="pss")
                nc.tensor.matmul(pssc[:1], ones_sb[:], acc_p[:],
                                 start=True, stop=True)
                # mask + exp on the partition-0 row, partial sum via accum
                sc_m = sc_pool.tile([1, ST], f32, tag="sc")
                off = b_p * S + si_p * ST
                nc.vector.tensor_tensor(sc_m[:], pssc[:1],
                                        mneg_row[:, off:off + ST], ALU.add)
                e_row = msk_pool.tile([1, ST], f32, tag="m")
                idx = b_p * NS + si_p
                nc.scalar.activation(e_row[:], sc_m[:], AF.Exp,
                                     accum_out=red_row[:, idx:idx + 1])
                nc.sync.dma_start(
                    e_sb[b_p:b_p + 1, si_p * ST:(si_p + 1) * ST], e_row[:])
                # scatter this tile's partial sum to its batch partition now
                nc.sync.dma_start(psums4[b_p:b_p + 1, si_p:si_p + 1],
                                  red_row[:, idx:idx + 1])

            pending = None
            # si-major so each score column-chunk [BL, ST] completes early
            # and its softmax piece overlaps the next chunk's matmuls.
            for si in range(NS):
                for b in range(BL):
                    if b == 0 and si == 0:
                        eo_sb = eo_first
                    else:
                        eo_sb = eo_pool.tile([P, KT, ST], bf16, tag="eo")
                        d_eo = nc.gpsimd.dma_start(
                            eo_sb[:],
                            eoT[b, :, si * ST:(si + 1) * ST].rearrange(
                                "(kk p) s -> p kk s", p=P))
                        if len(_dma_chain) < 4:
                            add_dep_helper(d_eo.ins, _dma_chain[-1].ins, True,
                                           "serial head dma")
                            _dma_chain.append(d_eo)
                    acc = acc_pool.tile([P, ST], bf16, tag="acc")
                    for hh in range(HT):
                        ps = psum_z.tile([P, ST], f32, tag="psz")
                        for kk in range(KT):
                            nc.tensor.matmul(
                                ps[:],
                                we_sb[:, kk, hh * P:(hh + 1) * P],
                                eo_sb[:, kk, :],
                                start=(kk == 0), stop=(kk == KT - 1))
                        if hh == 3:
                            flush_score(pending)
                            pending = None
                        t_sb = t_pool.tile([P, ST], bf16, tag="t")
                        nc.scalar.activation(
                            t_sb[:], ps[:], AF.Tanh,
                            bias=pre_sb[:, hh * BL + b:hh * BL + b + 1])
                        if hh == 0:
                            nc.vector.tensor_scalar(acc[:], t_sb[:],
                                                    v_sb[:, 0:1], None,
                                                    ALU.mult)
                        else:
                            tv = tv_pool.tile([P, ST], bf16, tag="tv")
                            nc.vector.tensor_scalar(tv[:], t_sb[:],
                                                    v_sb[:, hh:hh + 1], None,
                                                    ALU.mult)
                            nc.vector.tensor_tensor(acc[:], acc[:], tv[:],
                                                    ALU.add)
                    pending = (acc, b, si)

            flush_score(pending)

            # per-batch totals and reciprocal, already partition-aligned
            rinv4 = const.tile([BL, 1], f32)
            nc.vector.reduce_sum(rinv4[:], psums4[:], axis=AX.X)
            nc.vector.reciprocal(rinv4[:], rinv4[:])
            # normalize + store in two chunks so the DMA overlaps the mul
            o_sb = const.tile([BL, S], f32)
            for ci in range(2):
                cs = slice(ci * (S // 2), (ci + 1) * (S // 2))
                nc.vector.tensor_scalar(o_sb[:, cs], e_sb[:, cs], rinv4[:],
                                        None, ALU.mult)
                nc.sync.dma_start(out[:, cs], o_sb[:, cs])

    nc.compile()
    return nc


def _get_nc():
    global _compiled_nc
    if _compiled_nc is None:
        _compiled_nc = _build()
    return _compiled_nc


def _make_in_maps(hidden, encoder_outputs, encoder_mask, W, b, v):
    import ml_dtypes

    bf16 = ml_dtypes.bfloat16
    hidden = np.asarray(hidden, dtype=np.float32)
    encoder_outputs = np.asarray(encoder_outputs, dtype=np.float32)
    W = np.asarray(W, dtype=np.float32)
    b = np.asarray(b, dtype=np.float32)
    v = np.asarray(v, dtype=np.float32)
    mask_u8 = np.asarray(encoder_mask).reshape(B, S).astype(np.uint8)

    # [S, B, H] -> [B, H, S] so the contraction dim lands on partitions;
    # bf16 so the kernel streams half the bytes (matmuls run in bf16 anyway)
    eoT = np.ascontiguousarray(encoder_outputs.transpose(1, 2, 0)).astype(bf16)
    wT = np.ascontiguousarray(W.T).astype(bf16)         # [2H, H]
    bias_r = np.ascontiguousarray(b.reshape(HT, P).T)   # [P, HT]
    v_r = np.ascontiguousarray(v.reshape(HT, P).T)      # [P, HT]

    in_maps = []
    for c in range(NCORES):
        bs = slice(c * BL, (c + 1) * BL)
        h_c = hidden[bs]                                # [BL, H]
        hT_r = np.ascontiguousarray(
            h_c.T.reshape(KT, P, BL).transpose(1, 0, 2)).astype(bf16)
        in_maps.append({
            "eoT": eoT[bs],
            "wT": wT,
            "hTr": hT_r,
            "biasr": bias_r,
            "vr": v_r,
            "mask": mask_u8[bs],
        })
    return in_maps


def run(hidden, encoder_outputs, encoder_mask, W, b, v, trace=False):
    from concourse.bass_utils import run_bass_kernel_spmd

    nc = _get_nc()
    in_maps = _make_in_maps(hidden, encoder_outputs, encoder_mask, W, b, v)
    res = run_bass_kernel_spmd(nc, in_maps, core_ids=list(range(NCORES)),
                               trace=trace)
    out = np.concatenate([res.results[c]["out"] for c in range(NCORES)],
                         axis=0)
    return out.reshape(B, 1, S).astype(np.float32), res


def kernel(hidden, encoder_outputs, encoder_mask, W, b, v):
    out, _ = run(hidden, encoder_outputs, encoder_mask, W, b, v, trace=False)
    return out
